# revision 77
# baseline (speedup 1.0000x reference)
"""Trainium2 Bass kernel for nn_Model_34316788695805 (ragged_sequence).

Model: per-token char-level encoder GRU (C=8 steps) -> decoder GRU
(F=32 steps, teacher forced) -> vocab projection scores.

Sharding: token-parallel over 8 NeuronCores (32 tokens/core).  Each core
runs the full enc+dec GRU for its tokens and the full vocab projection,
producing a contiguous [1024, 10000] slab of the output.  No collectives;
the host concatenates the slabs.

Device layout: hidden dim H=1024 lives on partitions (8 chunks of 128),
tokens on the free axis.  GRU gate matmuls keep W_hh^T stationary
(fp8 e3m4 scaled x64 -> FWL loads 4 elems/read; the x64 is undone by
scale=1/64 inside the gate activations) and stream h (bf16).  Gate
arithmetic is fp32 on DVE/ACT, split in half-chunks so the chain
pipelines under the matmuls.  The vocab projection is a single batched
matmul at the end over all 32 steps (lhsT = transposed hidden states,
rhs = streamed out_W^T slabs).

Host-side work is limited to sharding/layout prep: embedding gathers,
mean-pooling of h0, weight transposes/casts, and EOS-freeze fixup
(a no-op for the generated inputs, which contain no EOS).
"""

import numpy as np
import ml_dtypes
from contextlib import ExitStack

import concourse.bass as bass
import concourse.mybir as mybir
import concourse.tile as tile
from concourse import bacc
from concourse.bass_utils import run_bass_kernel_spmd

# Problem constants (hardcoded per spec)
T, F, C, V, H, E, S = 256, 32, 8, 10000, 1024, 256, 512
PAD, BOS, EOS = 0, 1, 2
NCORES = 8
TC = T // NCORES          # 32 tokens per core
TS = TC * F               # 1024 (token,step) pairs per core
KH = H // 128             # 8 k-chunks of hidden
KE = E // 128             # 2 k-chunks of embedding
MG = H // 128             # 8 m-chunks per gate
VCH = 512                 # vocab chunk (one PSUM bank of fp32)
NV = (V + VCH - 1) // VCH  # 20 chunks
VPAD = NV * VCH           # 10240

F32 = mybir.dt.float32
BF16 = mybir.dt.bfloat16
F8E3 = mybir.dt.float8e3
AF = mybir.ActivationFunctionType
npbf16 = ml_dtypes.bfloat16
npe3m4 = ml_dtypes.float8_e3m4

WSCALE = 64.0       # fp8 W_hh pre-scale; undone by scale=1/WSCALE in the
SC = 1.0 / WSCALE   # gate activations (gi/biases are host-scaled to match)

_CACHE = {}


def _to_lhsT_layout(w):
    """[M, K] weight -> [128, K//128, M] array so that
    arr[p, k, m] = w[m, 128*k + p]; lhsT tile (k, m0) = arr[:, k, m0:m0+128]."""
    M, K = w.shape
    return np.ascontiguousarray(w.T.reshape(K // 128, 128, M).transpose(1, 0, 2))


def _cols_layout(x):
    """[N, K] -> [128, K//128, N]: arr[p, k, n] = x[n, 128*k + p] (rhs/moving)."""
    N, K = x.shape
    return np.ascontiguousarray(x.T.reshape(K // 128, 128, N).transpose(1, 0, 2))


def _build_program(flags):
    """Build + compile the Bacc/Tile program. flags: (gib_enc, ghn_enc,
    gib_dec, ghn_dec, outb) nonzero-bias booleans."""
    has_gib_enc, has_ghn_enc, has_gib_dec, has_ghn_dec, has_outb = flags

    nc = bacc.Bacc(
        "TRN2",
        target_bir_lowering=False,
        debug=False,
        enable_asserts=False,
        num_devices=NCORES,
    )

    # ---- DRAM I/O ----
    d_h0 = nc.dram_tensor("h0T", [128, KH, TC], F32, kind="ExternalInput").ap()
    d_xenc = nc.dram_tensor("xencT", [128, KE, C * TC], BF16, kind="ExternalInput").ap()
    d_xdec = nc.dram_tensor("xdecT", [128, KE, TS], BF16, kind="ExternalInput").ap()
    d_whh_e = nc.dram_tensor("whhTe", [128, KH, 3 * H], F8E3, kind="ExternalInput").ap()
    d_whh_d = nc.dram_tensor("whhTd", [128, KH, 3 * H], F8E3, kind="ExternalInput").ap()
    d_wih_e = nc.dram_tensor("wihTe", [128, KE, 3 * H], BF16, kind="ExternalInput").ap()
    d_wih_d = nc.dram_tensor("wihTd", [128, KE, 3 * H], BF16, kind="ExternalInput").ap()
    d_ow = nc.dram_tensor("owT", [NV, 128, KH, VCH], BF16, kind="ExternalInput").ap()
    d_gib_e = d_gib_d = d_ghn_e = d_ghn_d = d_outb = None
    d_gibrz_e = d_gibrz_d = None
    if has_gib_enc:
        d_gib_e = nc.dram_tensor("gibE", [128, 24], F32, kind="ExternalInput").ap()
        d_gibrz_e = nc.dram_tensor("gibrzE", [128, 2 * MG], F32, kind="ExternalInput").ap()
    if has_gib_dec:
        d_gib_d = nc.dram_tensor("gibD", [128, 24], F32, kind="ExternalInput").ap()
        d_gibrz_d = nc.dram_tensor("gibrzD", [128, 2 * MG], F32, kind="ExternalInput").ap()
    if has_ghn_enc:
        d_ghn_e = nc.dram_tensor("ghnE", [128, MG], F32, kind="ExternalInput").ap()
    if has_ghn_dec:
        d_ghn_d = nc.dram_tensor("ghnD", [128, MG], F32, kind="ExternalInput").ap()
    if has_outb:
        d_outb = nc.dram_tensor("outb", [1, VPAD], BF16, kind="ExternalInput").ap()
    d_scores = nc.dram_tensor("scores", [TS, V], BF16, kind="ExternalOutput").ap()

    GCH = 4  # steps per gi chunk
    JH = 4   # m-chunks per gi piece (one PSUM bank: 4*128 fp32 cols)
    with tile.TileContext(nc) as tc, ExitStack() as ctx:
        wpool = ctx.enter_context(tc.tile_pool(name="weights", bufs=1))
        whpool = ctx.enter_context(tc.tile_pool(name="whh", bufs=2))
        wipool = ctx.enter_context(tc.tile_pool(name="wih", bufs=1))
        gipool = ctx.enter_context(tc.tile_pool(name="gi", bufs=7))
        hpool = ctx.enter_context(tc.tile_pool(name="h", bufs=2))
        gpool = ctx.enter_context(tc.tile_pool(name="gates", bufs=2))
        spool = ctx.enter_context(tc.tile_pool(name="slab", bufs=8))
        stpool = ctx.enter_context(tc.tile_pool(name="staging", bufs=3))
        ps_gh = ctx.enter_context(tc.tile_pool(name="ps_gh", bufs=1, space="PSUM"))
        ps_gi = ctx.enter_context(tc.tile_pool(name="ps_gi", bufs=2, space="PSUM"))
        ps_sc = ctx.enter_context(tc.tile_pool(name="ps_sc", bufs=4, space="PSUM"))

        # ---- resident weights / inputs, spread across both HWDGE rings so
        # the startup loads run concurrently: sync carries h0 + W_hh slices
        # (feeds the first gh matmuls), ACT carries x_enc chunk 0 + W_ih
        # slices (feeds the first gi pieces, emitted mid-step-0) ----
        h_f = hpool.tile([128, KH, TC], F32, tag="hf")
        nc.sync.dma_start(h_f[:], d_h0)
        h_b = hpool.tile([128, KH, TC], BF16, tag="hb")
        nc.vector.tensor_copy(h_b[:], h_f[:])
        xenc = wpool.tile([128, KE, C * TC], BF16, tag="xenc")
        nc.scalar.dma_start(xenc[:, :, 0 : GCH * TC], d_xenc[:, :, 0 : GCH * TC])
        # gate-split DMAs in half-gate slices, gate order matching the
        # emission order in gru_step (r, n, z)
        wih_e = wipool.tile([128, KE, 3 * H], BF16, tag="wih")
        whh_e = whpool.tile([128, KH, 3 * H], F8E3, tag="whh")
        for g in (0, 2, 1):
            for m0 in (g * H, g * H + H // 2):
                nc.sync.dma_start(
                    whh_e[:, :, m0 : m0 + H // 2], d_whh_e[:, :, m0 : m0 + H // 2]
                )
                nc.scalar.dma_start(
                    wih_e[:, :, m0 : m0 + H // 2], d_wih_e[:, :, m0 : m0 + H // 2]
                )
        nc.scalar.dma_start(
            xenc[:, :, GCH * TC :], d_xenc[:, :, GCH * TC :]
        )
        xdec = wpool.tile([128, KE, TS], BF16, tag="xdec")
        nc.sync.dma_start(xdec[:], d_xdec)
        # hidden-state history (step-major columns: ts = s*TC + t), bf16;
        # the output DMA access pattern restores token-major row order
        hstT = wpool.tile([128, KH, F, TC], BF16, tag="hstT")

        gib_e = gib_d = ghn_e = ghn_d = gibrz_e = gibrz_d = None
        if has_gib_enc:
            gib_e = wpool.tile([128, 24], F32, tag="gib_e")
            nc.sync.dma_start(gib_e[:], d_gib_e)
            gibrz_e = wpool.tile([128, 2 * MG], F32, tag="gibrz_e")
            nc.sync.dma_start(gibrz_e[:], d_gibrz_e)
        if has_gib_dec:
            gib_d = wpool.tile([128, 24], F32, tag="gib_d")
            nc.sync.dma_start(gib_d[:], d_gib_d)
            gibrz_d = wpool.tile([128, 2 * MG], F32, tag="gibrz_d")
            nc.sync.dma_start(gibrz_d[:], d_gibrz_d)
        if has_ghn_enc:
            ghn_e = wpool.tile([128, MG], F32, tag="ghn_e")
            nc.sync.dma_start(ghn_e[:], d_ghn_e)
        if has_ghn_dec:
            ghn_d = wpool.tile([128, MG], F32, tag="ghn_d")
            nc.sync.dma_start(ghn_d[:], d_ghn_d)
        ones_row = None
        if has_outb:
            ones_row = wpool.tile([1, 128], BF16, tag="ones")
            nc.vector.memset(ones_row[:], 1.0)
        outb_sb = None
        if has_outb:
            outb_sb = wpool.tile([1, VPAD], BF16, tag="outb")
            nc.sync.dma_start(outb_sb[:], d_outb)

        def gi_piece(gi_tile, wih, x_ap, g, jh, gib, eng):
            """One bank-sized slab of gi = W_ih @ x: gate g, m-chunks
            [4*jh, 4*jh+4).  x_ap: [128, KE, GCH*TC] bf16.  Pieces are
            emitted spread across the preceding steps so the single wide
            PSUM->SBUF copy (alternating DVE/ACT via `eng`) never queues
            ahead of the gate-chain ops."""
            n = GCH * TC
            ps = ps_gi.tile([128, JH, n], F32, tag="ps_gi")
            for jj in range(JH):
                m = g * H + (jh * JH + jj) * 128
                for k in range(KE):
                    nc.tensor.matmul(
                        ps[:, jj, :],
                        wih[:, k, m : m + 128],
                        x_ap[:, k, :],
                        start=(k == 0),
                        stop=(k == KE - 1),
                    )
            dst = gi_tile[:, g, jh * JH : (jh + 1) * JH, :n]
            if gib is not None:
                for jj in range(JH):
                    j = jh * JH + jj
                    nc.scalar.activation(
                        gi_tile[:, g, j, :n], ps[:, jj, :], AF.Identity,
                        bias=gib[:, g * MG + j : g * MG + j + 1],
                    )
            elif eng == 0:
                nc.vector.tensor_copy(dst, ps[:])
            else:
                nc.scalar.copy(dst, ps[:])

        def gi_pieces(gi_tile, wih, x_ap, gib):
            return [
                (lambda g=g, jh=jh, i=i: gi_piece(
                    gi_tile, wih, x_ap, g, jh, gib, i % 2))
                for i, (g, jh) in enumerate(
                    (g, jh) for g in range(3) for jh in range(MG // JH))
            ]

        def gru_step(whh, gi_tile, s_in_chunk, ghn, hst_write_step=None,
                     mid_fill=None):
            """One GRU step: h (h_b tile, closed over) -> new h.
            Gate order r, n, z: the r/n chains (half-chunks, so each half
            starts as soon as its PSUM columns land) pipeline under the
            remaining matmuls; the z tail (add->sigmoid->mul->add,
            full-width: fewer serial hops) trails the last matmul by
            ~2us, covered by the interleaved scores/gi work.  All
            pre-activations are x WSCALE (fp8 weight scaling); the
            activations undo it via scale=SC."""
            nonlocal h_b
            c0 = s_in_chunk * TC
            gh_r = ps_gh.tile([128, MG, TC], F32, tag="gh_r")
            gh_n = ps_gh.tile([128, MG, TC], F32, tag="gh_n")
            for g, ps in ((0, gh_r), (2, gh_n)):
                for j in range(MG):
                    m = g * H + j * 128
                    for k in range(KH):
                        nc.tensor.matmul(
                            ps[:, j, :],
                            whh[:, k, m : m + 128],
                            h_b[:, k, :],
                            start=(k == 0),
                            stop=(k == KH - 1),
                        )
            # fill work (gi pieces) goes BETWEEN the n and z matmul groups:
            # the PE sem increments serialize at ~26ns each and lag a burst
            # of back-to-back 32-col matmuls by ~1-2us, so the z tail would
            # otherwise start that late; a few wide matmuls here let the
            # counter catch up before the z group whose completion gates it
            if mid_fill is not None:
                mid_fill()
            # z reuses r's PSUM bank (r_pre consumes gh_r early); frees a
            # bank so ps_sc can run 4-deep
            gh_z = ps_gh.tile([128, MG, TC], F32, tag="gh_r")
            for j in range(MG):
                m = H + j * 128
                for k in range(KH):
                    nc.tensor.matmul(
                        gh_z[:, j, :],
                        whh[:, k, m : m + 128],
                        h_b[:, k, :],
                        start=(k == 0),
                        stop=(k == KH - 1),
                    )
            gi_r = gi_tile[:, 0, :, c0 : c0 + TC]
            gi_z = gi_tile[:, 1, :, c0 : c0 + TC]
            gi_n = gi_tile[:, 2, :, c0 : c0 + TC]

            halves = (slice(0, MG // 2), slice(MG // 2, MG))
            r_pre = gpool.tile([128, MG, TC], F32, tag="r_pre")
            r = gpool.tile([128, MG, TC], F32, tag="r")
            ghn_sb = gpool.tile([128, MG, TC], F32, tag="ghn_sb") if ghn is not None else None
            rn = gpool.tile([128, MG, TC], F32, tag="rn")
            n_pre = gpool.tile([128, MG, TC], F32, tag="n_pre")
            n = gpool.tile([128, MG, TC], F32, tag="n")
            d = gpool.tile([128, MG, TC], F32, tag="d")
            # reuses r_pre's buffer (consumed early by sigmoid(r))
            z_pre = gpool.tile([128, MG, TC], F32, tag="r_pre")
            z = gpool.tile([128, MG, TC], F32, tag="z")
            # reuses rn's buffer (consumed mid-step by the n_pre add)
            zd = gpool.tile([128, MG, TC], F32, tag="rn")
            # decoder steps write h (bf16) straight into the history tile
            if hst_write_step is not None:
                nh_b = hstT[:, :, hst_write_step, :]
            else:
                nh_b_t = hpool.tile([128, KH, TC], BF16, tag="hb")
                nh_b = nh_b_t[:]
            for X in halves:
                nc.vector.tensor_add(r_pre[:, X, :], gi_r[:, X, :], gh_r[:, X, :])
            for X in halves:
                nc.scalar.activation(r[:, X, :], r_pre[:, X, :], AF.Sigmoid, scale=SC)
            if ghn is not None:
                for j in range(MG):
                    nc.scalar.activation(
                        ghn_sb[:, j, :], gh_n[:, j, :], AF.Identity,
                        bias=ghn[:, j : j + 1],
                    )
                n_src = ghn_sb
            else:
                n_src = gh_n
            for X in halves:
                nc.vector.tensor_mul(rn[:, X, :], r[:, X, :], n_src[:, X, :])
            for X in halves:
                nc.vector.tensor_add(n_pre[:, X, :], rn[:, X, :], gi_n[:, X, :])
            for X in halves:
                nc.scalar.activation(n[:, X, :], n_pre[:, X, :], AF.Tanh, scale=SC)
            for X in halves:
                nc.vector.tensor_sub(d[:, X, :], h_b[:, X, :], n[:, X, :])
            # z tail, full-width: add -> sigmoid -> mul -> adds
            nc.vector.tensor_add(z_pre[:], gi_z, gh_z[:])
            nc.scalar.activation(z[:], z_pre[:], AF.Sigmoid, scale=SC)
            nc.vector.tensor_mul(zd[:], z[:], d[:])
            # h (bf16) in halves: the next step's matmuls consume k-chunks
            # 0-3 first, so they start as soon as the first half lands
            for X in halves:
                nc.vector.tensor_add(nh_b[:, X, :], n[:, X, :], zd[:, X, :])
            h_b = nh_b

        # ---- encoder ----
        from collections import deque

        # chunk-0 gi pieces drain inside step 0 (mid_fill), after the r/n
        # gh matmuls: those only need h0 + the sync ring's first W_hh
        # slices, so the PE ramps ~4us earlier than if pieces ran first
        piece_q = deque()
        gi_t = gipool.tile([128, 3, MG, GCH * TC], BF16, tag="gi")
        piece_q.extend(gi_pieces(gi_t, wih_e, xenc[:, :, 0 : GCH * TC], gib_e))
        def drain_pieces(k=2):
            for _ in range(k):
                if piece_q:
                    piece_q.popleft()()

        dec_chunks = []
        for s in range(C):
            g = s // GCH
            if s % GCH == 0 and g + 1 < C // GCH:
                gi_next = gipool.tile([128, 3, MG, GCH * TC], BF16, tag="gi")
                piece_q.extend(gi_pieces(
                    gi_next, wih_e,
                    xenc[:, :, (g + 1) * GCH * TC : (g + 2) * GCH * TC],
                    gib_e,
                ))
            # decoder chunks 0-3 fill the encoder steps' idle (the ~3us h
            # chain has no scores work to hide under here); bf16 gi tiles
            # keep 5 chunks alive in SBUF
            if 0 < s <= 6 and s != 5:
                dc = {1: 0, 2: 1, 3: 2, 4: 3, 6: 4}[s]
                gi_d = gipool.tile([128, 3, MG, GCH * TC], BF16, tag="gi")
                piece_q.extend(gi_pieces(
                    gi_d, wih_d,
                    xdec[:, :, dc * GCH * TC : (dc + 1) * GCH * TC], gib_d))
                dec_chunks.append(gi_d)
            gru_step(whh_e, gi_t, s % GCH, ghn_e,
                     mid_fill=(lambda: drain_pieces(6)) if s == 0 else None)
            drain_pieces(4)
            if s == 0:
                # dec weights load during the encoder (DMA is idle here)
                wih_d = wipool.tile([128, KE, 3 * H], BF16, tag="wih")
                nc.scalar.dma_start(wih_d[:], d_wih_d)
                whh_d = whpool.tile([128, KH, 3 * H], F8E3, tag="whh")
                nc.sync.dma_start(whh_d[:], d_whh_d)
            if s % GCH == GCH - 1 and g + 1 < C // GCH:
                gi_t = gi_next

        # output rows are stored step-major (row = s*TC + t, contiguous
        # per block); the host reorders rows to token-major afterwards
        def scores_block(sb, c, slab, par):
            """Scores for step block sb (4 steps) x vocab chunk c."""
            ncols = min(VCH, V - c * VCH)
            ps = ps_sc.tile([128, VCH], F32, tag="ps_sc")
            for k in range(KH):
                nc.tensor.matmul(
                    ps[:, :ncols],
                    hstT[:, k, 4 * sb : 4 * sb + 4, :],
                    slab[:, k, :ncols],
                    start=(k == 0),
                    stop=False if has_outb else (k == KH - 1),
                )
            if has_outb:
                nc.tensor.matmul(
                    ps[:, :ncols], ones_row[:],
                    outb_sb[:, c * VCH : c * VCH + ncols],
                    start=False, stop=True,
                )
            st = stpool.tile([128, VCH], BF16, tag="st")
            nc.scalar.copy(st[:, :ncols], ps[:, :ncols])
            # stores on the ACT HWDGE ring; slab loads stay on SP's
            nc.scalar.dma_start(
                d_scores[128 * sb : 128 * (sb + 1), c * VCH : c * VCH + ncols],
                st[:, :ncols],
            )

        # ---- decoder (scores for the first vocab chunks fill step tails) ----
        N_INTER = 8
        inter_slabs = []
        for c in range(N_INTER):
            slab = spool.tile([128, KH, VCH], BF16, tag="slab")
            nc.sync.dma_start(slab[:], d_ow[c])
            inter_slabs.append(slab)

        pending = deque()
        gi_t = dec_chunks[0]
        gi_ready = deque(dec_chunks[1:])
        for s in range(F):
            g = s // GCH
            if s % GCH == 0 and g + 5 < F // GCH:
                gi_next = gipool.tile([128, 3, MG, GCH * TC], BF16, tag="gi")
                piece_q.extend(gi_pieces(
                    gi_next, wih_d,
                    xdec[:, :, (g + 5) * GCH * TC : (g + 6) * GCH * TC],
                    gib_d,
                ))
                gi_ready.append(gi_next)
            gru_step(whh_d, gi_t, s % GCH, ghn_d, hst_write_step=s)
            drain_pieces()
            if s % GCH == GCH - 1 and g + 1 < F // GCH:
                gi_t = gi_ready.popleft()
            # scores blocks from ALREADY-COMPLETE step blocks fill the step
            # tails; 1.5/step keeps the PE work-bound past the ~3us h chain
            for _ in range(2):
                if pending:
                    sb, c = pending.popleft()
                    scores_block(sb, c, inter_slabs[c], sb + c)
            if s % 4 == 3:
                pending.extend((s // 4, c) for c in range(N_INTER))
        for sb, c in pending:
            scores_block(sb, c, inter_slabs[c], sb + c)

        # ---- remaining vocab projection ----
        for c in range(N_INTER, NV):
            slab = spool.tile([128, KH, VCH], BF16, tag="slab")
            nc.sync.dma_start(slab[:], d_ow[c])
            for sb in range(F // 4):
                scores_block(sb, c, slab, sb + c)

    nc.compile()
    return nc


def _prep_inputs(token_ctx, char_emb_w, enc_W_ih, enc_W_hh, enc_b_ih, enc_b_hh,
                 dec_W_ih, dec_W_hh, dec_b_ih, dec_b_hh, out_W, out_b,
                 in_sent_token_chars, out_chars):
    """Host-side sharding/layout prep. Returns (in_maps, flags, fixup_info)."""
    tcarr = np.asarray(in_sent_token_chars)[0].reshape(T, C, 3)
    chars = tcarr[:, :, 2]
    xt = tcarr[:, :, 1]
    token_ctx = np.asarray(token_ctx)[0]          # [S, H]
    char_emb_w = np.asarray(char_emb_w)           # [V, E]
    out_chars = np.asarray(out_chars)[0]          # [1 + T*F]

    h0 = token_ctx[xt].mean(axis=1).astype(np.float32)      # [T, H]
    x_enc = char_emb_w[chars]                                # [T, C, E]
    gold = out_chars[1 : 1 + T * F].reshape(T, F)
    c0 = out_chars[0]
    c_in = np.concatenate(
        [np.full((T, 1), c0, dtype=gold.dtype), gold[:, :-1]], axis=1
    )                                                        # [T, F]
    x_dec = char_emb_w[c_in]                                 # [T, F, E]

    # shared (replicated) weight layouts; W_hh is fp8 e3m4 scaled x64 (the
    # gate activations undo it), so W_ih/biases are pre-scaled to match
    whhTe = _to_lhsT_layout(np.asarray(enc_W_hh) * WSCALE).astype(npe3m4)
    whhTd = _to_lhsT_layout(np.asarray(dec_W_hh) * WSCALE).astype(npe3m4)
    wihTe = _to_lhsT_layout(np.asarray(enc_W_ih) * WSCALE).astype(npbf16)
    wihTd = _to_lhsT_layout(np.asarray(dec_W_ih) * WSCALE).astype(npbf16)
    owpad = np.zeros((VPAD, H), np.float32)
    owpad[:V] = np.asarray(out_W)
    owT = np.ascontiguousarray(
        owpad.reshape(NV, VCH, KH, 128).transpose(0, 3, 2, 1)
    ).astype(npbf16)                                          # [NV,128,KH,VCH]

    def gate_bias(b_ih, b_hh):
        # gib/ghn live in the x WSCALE PSUM domain; gibrz (r/z fold path) is
        # applied AFTER the activation's scale=1/WSCALE, so it stays unscaled
        b_ih = np.asarray(b_ih); b_hh = np.asarray(b_hh)
        grz = b_ih[: 2 * H] + b_hh[: 2 * H]
        gib = np.concatenate([grz, b_ih[2 * H :]]) * WSCALE
        ghn = b_hh[2 * H :] * WSCALE
        gib_l = np.ascontiguousarray(gib.reshape(24, 128).T).astype(np.float32)
        ghn_l = np.ascontiguousarray(ghn.reshape(MG, 128).T).astype(np.float32)
        grz_l = np.ascontiguousarray(grz.reshape(2 * MG, 128).T).astype(np.float32)
        return gib_l, ghn_l, grz_l, bool(np.any(gib)), bool(np.any(ghn))

    gibE, ghnE, grzE, has_gib_e, has_ghn_e = gate_bias(enc_b_ih, enc_b_hh)
    gibD, ghnD, grzD, has_gib_d, has_ghn_d = gate_bias(dec_b_ih, dec_b_hh)
    out_b = np.asarray(out_b)
    has_outb = bool(np.any(out_b))
    outb_pad = np.zeros((1, VPAD), npbf16)
    outb_pad[0, :V] = out_b.astype(npbf16)

    flags = (has_gib_e, has_ghn_e, has_gib_d, has_ghn_d, has_outb)

    in_maps = []
    for ci in range(NCORES):
        sl = slice(ci * TC, (ci + 1) * TC)
        h0T = np.ascontiguousarray(
            h0[sl].T.reshape(KH, 128, TC).transpose(1, 0, 2)
        )
        # enc ts = c*TC + t (step-major)
        xe = x_enc[sl].transpose(1, 0, 2).reshape(C * TC, E).astype(np.float32)
        xencT = _cols_layout(xe).astype(npbf16)
        # dec ts = s*TC + t (step-major)
        xd = x_dec[sl].transpose(1, 0, 2).reshape(TS, E).astype(np.float32)
        xdecT = _cols_layout(xd).astype(npbf16)
        m = {
            "h0T": h0T, "xencT": xencT, "xdecT": xdecT,
            "whhTe": whhTe, "whhTd": whhTd, "wihTe": wihTe, "wihTd": wihTd,
            "owT": owT,
        }
        if has_gib_e: m["gibE"] = gibE; m["gibrzE"] = grzE
        if has_gib_d: m["gibD"] = gibD; m["gibrzD"] = grzD
        if has_ghn_e: m["ghnE"] = ghnE
        if has_ghn_d: m["ghnD"] = ghnD
        if has_outb: m["outb"] = outb_pad
        in_maps.append(m)

    return in_maps, flags, (gold, c0)


def _eos_fixup(scores, gold, c0):
    """Apply the reference's EOS freeze/pad semantics on the host.
    scores: [T, F, V] (modified in place)."""
    if c0 != EOS and not np.any(gold == EOS):
        return scores
    done0 = c0 == EOS
    for t in range(T):
        hits = np.nonzero(gold[t] == EOS)[0]
        if done0:
            first_done = 0
        elif len(hits):
            first_done = int(hits[0]) + 1
        else:
            continue
        if first_done == 0:
            scores[t, :, :] = 0.0
        elif first_done < F:
            scores[t, first_done:, :] = scores[t, first_done - 1, :]
    return scores


def kernel(**inputs) -> np.ndarray:
    assert int(inputs["max_tokens"]) == T
    assert int(inputs["max_form_len"]) == F
    assert int(inputs["use_teacher_forcing"]) == 1

    in_maps, flags, (gold, c0) = _prep_inputs(
        inputs["token_ctx"], inputs["char_emb_w"],
        inputs["enc_W_ih"], inputs["enc_W_hh"], inputs["enc_b_ih"], inputs["enc_b_hh"],
        inputs["dec_W_ih"], inputs["dec_W_hh"], inputs["dec_b_ih"], inputs["dec_b_hh"],
        inputs["out_W"], inputs["out_b"],
        inputs["in_sent_token_chars"], inputs["out_chars"],
    )

    if flags not in _CACHE:
        _CACHE[flags] = _build_program(flags)
    nc = _CACHE[flags]

    trace = bool(_RUN_OPTS.get("trace"))
    res = run_bass_kernel_spmd(
        nc, in_maps, core_ids=list(range(NCORES)), trace=trace,
        **_RUN_OPTS.get("kwargs", {}),
    )
    _RUN_OPTS["last_result"] = res

    # device rows are step-major per core; reorder to token-major
    slabs = [
        res.results[ci]["scores"].astype(np.float32).reshape(F, TC, V).transpose(1, 0, 2)
        for ci in range(NCORES)
    ]
    out = np.concatenate(slabs, axis=0)  # [T, F, V]
    out = _eos_fixup(out, gold, c0)
    return np.ascontiguousarray(out.reshape(1, T * F, V))


# knobs used by test.py (harness just calls kernel())
_RUN_OPTS = {"trace": False, "kwargs": {}}



# revision 78
# speedup vs baseline: 1.0087x; 1.0087x over previous
"""Trainium2 Bass kernel for nn_Model_34316788695805 (ragged_sequence).

Model: per-token char-level encoder GRU (C=8 steps) -> decoder GRU
(F=32 steps, teacher forced) -> vocab projection scores.

Sharding: token-parallel over 8 NeuronCores (32 tokens/core).  Each core
runs the full enc+dec GRU for its tokens and the full vocab projection,
producing a contiguous [1024, 10000] slab of the output.  No collectives;
the host concatenates the slabs.

Device layout: hidden dim H=1024 lives on partitions (8 chunks of 128),
tokens on the free axis.  GRU gate matmuls keep W_hh^T stationary
(fp8 e3m4 scaled x64 -> FWL loads 4 elems/read; the x64 is undone by
scale=1/64 inside the gate activations) and stream h (bf16).  Gate
arithmetic is fp32 on DVE/ACT, split in half-chunks so the chain
pipelines under the matmuls.  The vocab projection is a single batched
matmul at the end over all 32 steps (lhsT = transposed hidden states,
rhs = streamed out_W^T slabs).

Host-side work is limited to sharding/layout prep: embedding gathers,
mean-pooling of h0, weight transposes/casts, and EOS-freeze fixup
(a no-op for the generated inputs, which contain no EOS).
"""

import numpy as np
import ml_dtypes
from contextlib import ExitStack

import concourse.bass as bass
import concourse.mybir as mybir
import concourse.tile as tile
from concourse import bacc
from concourse.bass_utils import run_bass_kernel_spmd

# Problem constants (hardcoded per spec)
T, F, C, V, H, E, S = 256, 32, 8, 10000, 1024, 256, 512
PAD, BOS, EOS = 0, 1, 2
NCORES = 8
TC = T // NCORES          # 32 tokens per core
TS = TC * F               # 1024 (token,step) pairs per core
KH = H // 128             # 8 k-chunks of hidden
KE = E // 128             # 2 k-chunks of embedding
MG = H // 128             # 8 m-chunks per gate
VCH = 512                 # vocab chunk (one PSUM bank of fp32)
NV = (V + VCH - 1) // VCH  # 20 chunks
VPAD = NV * VCH           # 10240

F32 = mybir.dt.float32
BF16 = mybir.dt.bfloat16
F8E3 = mybir.dt.float8e3
AF = mybir.ActivationFunctionType
npbf16 = ml_dtypes.bfloat16
npe3m4 = ml_dtypes.float8_e3m4

WSCALE = 64.0       # fp8 W_hh pre-scale; undone by scale=1/WSCALE in the
SC = 1.0 / WSCALE   # gate activations (gi/biases are host-scaled to match)

_CACHE = {}


def _to_lhsT_layout(w):
    """[M, K] weight -> [128, K//128, M] array so that
    arr[p, k, m] = w[m, 128*k + p]; lhsT tile (k, m0) = arr[:, k, m0:m0+128]."""
    M, K = w.shape
    return np.ascontiguousarray(w.T.reshape(K // 128, 128, M).transpose(1, 0, 2))


def _cols_layout(x):
    """[N, K] -> [128, K//128, N]: arr[p, k, n] = x[n, 128*k + p] (rhs/moving)."""
    N, K = x.shape
    return np.ascontiguousarray(x.T.reshape(K // 128, 128, N).transpose(1, 0, 2))


def _build_program(flags):
    """Build + compile the Bacc/Tile program. flags: (gib_enc, ghn_enc,
    gib_dec, ghn_dec, outb) nonzero-bias booleans."""
    has_gib_enc, has_ghn_enc, has_gib_dec, has_ghn_dec, has_outb = flags

    nc = bacc.Bacc(
        "TRN2",
        target_bir_lowering=False,
        debug=False,
        enable_asserts=False,
        num_devices=NCORES,
    )

    # ---- DRAM I/O ----
    d_h0 = nc.dram_tensor("h0T", [128, KH, TC], F32, kind="ExternalInput").ap()
    d_xenc = nc.dram_tensor("xencT", [128, KE, C * TC], BF16, kind="ExternalInput").ap()
    d_xdec = nc.dram_tensor("xdecT", [128, KE, TS], BF16, kind="ExternalInput").ap()
    d_whh_e = nc.dram_tensor("whhTe", [128, KH, 3 * H], F8E3, kind="ExternalInput").ap()
    d_whh_d = nc.dram_tensor("whhTd", [128, KH, 3 * H], F8E3, kind="ExternalInput").ap()
    d_wih_e = nc.dram_tensor("wihTe", [128, KE, 3 * H], BF16, kind="ExternalInput").ap()
    d_wih_d = nc.dram_tensor("wihTd", [128, KE, 3 * H], BF16, kind="ExternalInput").ap()
    d_ow = nc.dram_tensor("owT", [NV, 128, KH, VCH], BF16, kind="ExternalInput").ap()
    d_gib_e = d_gib_d = d_ghn_e = d_ghn_d = d_outb = None
    d_gibrz_e = d_gibrz_d = None
    if has_gib_enc:
        d_gib_e = nc.dram_tensor("gibE", [128, 24], F32, kind="ExternalInput").ap()
        d_gibrz_e = nc.dram_tensor("gibrzE", [128, 2 * MG], F32, kind="ExternalInput").ap()
    if has_gib_dec:
        d_gib_d = nc.dram_tensor("gibD", [128, 24], F32, kind="ExternalInput").ap()
        d_gibrz_d = nc.dram_tensor("gibrzD", [128, 2 * MG], F32, kind="ExternalInput").ap()
    if has_ghn_enc:
        d_ghn_e = nc.dram_tensor("ghnE", [128, MG], F32, kind="ExternalInput").ap()
    if has_ghn_dec:
        d_ghn_d = nc.dram_tensor("ghnD", [128, MG], F32, kind="ExternalInput").ap()
    if has_outb:
        d_outb = nc.dram_tensor("outb", [1, VPAD], BF16, kind="ExternalInput").ap()
    d_scores = nc.dram_tensor("scores", [TS, V], BF16, kind="ExternalOutput").ap()

    GCH = 4  # steps per gi chunk
    JH = 4   # m-chunks per gi piece (one PSUM bank: 4*128 fp32 cols)
    with tile.TileContext(nc) as tc, ExitStack() as ctx:
        wpool = ctx.enter_context(tc.tile_pool(name="weights", bufs=1))
        whpool = ctx.enter_context(tc.tile_pool(name="whh", bufs=2))
        wipool = ctx.enter_context(tc.tile_pool(name="wih", bufs=1))
        gipool = ctx.enter_context(tc.tile_pool(name="gi", bufs=7))
        hpool = ctx.enter_context(tc.tile_pool(name="h", bufs=2))
        gpool = ctx.enter_context(tc.tile_pool(name="gates", bufs=2))
        spool = ctx.enter_context(tc.tile_pool(name="slab", bufs=8))
        stpool = ctx.enter_context(tc.tile_pool(name="staging", bufs=3))
        ps_gh = ctx.enter_context(tc.tile_pool(name="ps_gh", bufs=1, space="PSUM"))
        ps_gi = ctx.enter_context(tc.tile_pool(name="ps_gi", bufs=2, space="PSUM"))
        ps_sc = ctx.enter_context(tc.tile_pool(name="ps_sc", bufs=4, space="PSUM"))

        # ---- resident weights / inputs, spread across both HWDGE rings so
        # the startup loads run concurrently: sync carries h0 + W_hh slices
        # (feeds the first gh matmuls), ACT carries x_enc chunk 0 + W_ih
        # slices (feeds the first gi pieces, emitted mid-step-0) ----
        h_f = hpool.tile([128, KH, TC], F32, tag="hf")
        nc.sync.dma_start(h_f[:], d_h0)
        h_b = hpool.tile([128, KH, TC], BF16, tag="hb")
        nc.vector.tensor_copy(h_b[:], h_f[:])
        xenc = wpool.tile([128, KE, C * TC], BF16, tag="xenc")
        nc.scalar.dma_start(xenc[:, :, 0 : GCH * TC], d_xenc[:, :, 0 : GCH * TC])
        # gate-split DMAs in half-gate slices, gate order matching the
        # emission order in gru_step (r, n, z)
        wih_e = wipool.tile([128, KE, 3 * H], BF16, tag="wih")
        whh_e = whpool.tile([128, KH, 3 * H], F8E3, tag="whh")
        for g in (0, 2, 1):
            for m0 in (g * H, g * H + H // 2):
                nc.sync.dma_start(
                    whh_e[:, :, m0 : m0 + H // 2], d_whh_e[:, :, m0 : m0 + H // 2]
                )
                nc.scalar.dma_start(
                    wih_e[:, :, m0 : m0 + H // 2], d_wih_e[:, :, m0 : m0 + H // 2]
                )
        nc.scalar.dma_start(
            xenc[:, :, GCH * TC :], d_xenc[:, :, GCH * TC :]
        )
        xdec = wpool.tile([128, KE, TS], BF16, tag="xdec")
        nc.sync.dma_start(xdec[:], d_xdec)
        # hidden-state history (step-major columns: ts = s*TC + t), bf16;
        # the output DMA access pattern restores token-major row order
        hstT = wpool.tile([128, KH, F, TC], BF16, tag="hstT")

        gib_e = gib_d = ghn_e = ghn_d = gibrz_e = gibrz_d = None
        if has_gib_enc:
            gib_e = wpool.tile([128, 24], F32, tag="gib_e")
            nc.sync.dma_start(gib_e[:], d_gib_e)
            gibrz_e = wpool.tile([128, 2 * MG], F32, tag="gibrz_e")
            nc.sync.dma_start(gibrz_e[:], d_gibrz_e)
        if has_gib_dec:
            gib_d = wpool.tile([128, 24], F32, tag="gib_d")
            nc.sync.dma_start(gib_d[:], d_gib_d)
            gibrz_d = wpool.tile([128, 2 * MG], F32, tag="gibrz_d")
            nc.sync.dma_start(gibrz_d[:], d_gibrz_d)
        if has_ghn_enc:
            ghn_e = wpool.tile([128, MG], F32, tag="ghn_e")
            nc.sync.dma_start(ghn_e[:], d_ghn_e)
        if has_ghn_dec:
            ghn_d = wpool.tile([128, MG], F32, tag="ghn_d")
            nc.sync.dma_start(ghn_d[:], d_ghn_d)
        ones_row = None
        if has_outb:
            ones_row = wpool.tile([1, 128], BF16, tag="ones")
            nc.vector.memset(ones_row[:], 1.0)
        outb_sb = None
        if has_outb:
            outb_sb = wpool.tile([1, VPAD], BF16, tag="outb")
            nc.sync.dma_start(outb_sb[:], d_outb)

        def gi_piece(gi_tile, wih, x_ap, g, jh, gib, eng):
            """One bank-sized slab of gi = W_ih @ x: gate g, m-chunks
            [4*jh, 4*jh+4).  x_ap: [128, KE, GCH*TC] bf16.  Pieces are
            emitted spread across the preceding steps so the single wide
            PSUM->SBUF copy (alternating DVE/ACT via `eng`) never queues
            ahead of the gate-chain ops."""
            n = GCH * TC
            ps = ps_gi.tile([128, JH, n], F32, tag="ps_gi")
            for jj in range(JH):
                m = g * H + (jh * JH + jj) * 128
                for k in range(KE):
                    nc.tensor.matmul(
                        ps[:, jj, :],
                        wih[:, k, m : m + 128],
                        x_ap[:, k, :],
                        start=(k == 0),
                        stop=(k == KE - 1),
                    )
            dst = gi_tile[:, g, jh * JH : (jh + 1) * JH, :n]
            if gib is not None:
                for jj in range(JH):
                    j = jh * JH + jj
                    nc.scalar.activation(
                        gi_tile[:, g, j, :n], ps[:, jj, :], AF.Identity,
                        bias=gib[:, g * MG + j : g * MG + j + 1],
                    )
            elif eng == 0:
                nc.vector.tensor_copy(dst, ps[:])
            else:
                nc.scalar.copy(dst, ps[:])

        def gi_pieces(gi_tile, wih, x_ap, gib):
            return [
                (lambda g=g, jh=jh, i=i: gi_piece(
                    gi_tile, wih, x_ap, g, jh, gib, i % 2))
                for i, (g, jh) in enumerate(
                    (g, jh) for g in range(3) for jh in range(MG // JH))
            ]

        def gru_step(whh, gi_tile, s_in_chunk, ghn, hst_write_step=None,
                     mid_fill=None):
            """One GRU step: h (h_f/h_b tiles, closed over) -> new h.
            Gate order r, n, z: the r/n chains (half-chunks, so each half
            starts as soon as its PSUM columns land) pipeline under the
            remaining matmuls; the z tail (add->sigmoid->mul->add,
            full-width: fewer serial hops) trails the last matmul by
            ~2us, covered by the interleaved scores/gi work.  All
            pre-activations are x WSCALE (fp8 weight scaling); the
            activations undo it via scale=SC."""
            nonlocal h_f, h_b
            c0 = s_in_chunk * TC
            gh_r = ps_gh.tile([128, MG, TC], F32, tag="gh_r")
            gh_n = ps_gh.tile([128, MG, TC], F32, tag="gh_n")
            for g, ps in ((0, gh_r), (2, gh_n)):
                for j in range(MG):
                    m = g * H + j * 128
                    for k in range(KH):
                        nc.tensor.matmul(
                            ps[:, j, :],
                            whh[:, k, m : m + 128],
                            h_b[:, k, :],
                            start=(k == 0),
                            stop=(k == KH - 1),
                        )
            # fill work (gi pieces) goes BETWEEN the n and z matmul groups:
            # the PE sem increments serialize at ~26ns each and lag a burst
            # of back-to-back 32-col matmuls by ~1-2us, so the z tail would
            # otherwise start that late; a few wide matmuls here let the
            # counter catch up before the z group whose completion gates it
            if mid_fill is not None:
                mid_fill()
            # z reuses r's PSUM bank (r_pre consumes gh_r early); frees a
            # bank so ps_sc can run 4-deep
            gh_z = ps_gh.tile([128, MG, TC], F32, tag="gh_r")
            for j in range(MG):
                m = H + j * 128
                for k in range(KH):
                    nc.tensor.matmul(
                        gh_z[:, j, :],
                        whh[:, k, m : m + 128],
                        h_b[:, k, :],
                        start=(k == 0),
                        stop=(k == KH - 1),
                    )
            gi_r = gi_tile[:, 0, :, c0 : c0 + TC]
            gi_z = gi_tile[:, 1, :, c0 : c0 + TC]
            gi_n = gi_tile[:, 2, :, c0 : c0 + TC]

            halves = (slice(0, MG // 2), slice(MG // 2, MG))
            r_pre = gpool.tile([128, MG, TC], F32, tag="r_pre")
            r = gpool.tile([128, MG, TC], F32, tag="r")
            ghn_sb = gpool.tile([128, MG, TC], F32, tag="ghn_sb") if ghn is not None else None
            rn = gpool.tile([128, MG, TC], F32, tag="rn")
            n_pre = gpool.tile([128, MG, TC], F32, tag="n_pre")
            n = gpool.tile([128, MG, TC], F32, tag="n")
            d = gpool.tile([128, MG, TC], F32, tag="d")
            # reuses r_pre's buffer (consumed early by sigmoid(r))
            z_pre = gpool.tile([128, MG, TC], F32, tag="r_pre")
            z = gpool.tile([128, MG, TC], F32, tag="z")
            # reuses rn's buffer (consumed mid-step by the n_pre add)
            zd = gpool.tile([128, MG, TC], F32, tag="rn")
            # decoder steps write h (bf16) straight into the history tile
            if hst_write_step is not None:
                nh_b = hstT[:, :, hst_write_step, :]
            else:
                nh_b_t = hpool.tile([128, KH, TC], BF16, tag="hb")
                nh_b = nh_b_t[:]
            nh_f = hpool.tile([128, KH, TC], F32, tag="hf")
            for X in halves:
                nc.vector.tensor_add(r_pre[:, X, :], gi_r[:, X, :], gh_r[:, X, :])
            for X in halves:
                nc.scalar.activation(r[:, X, :], r_pre[:, X, :], AF.Sigmoid, scale=SC)
            if ghn is not None:
                for j in range(MG):
                    nc.scalar.activation(
                        ghn_sb[:, j, :], gh_n[:, j, :], AF.Identity,
                        bias=ghn[:, j : j + 1],
                    )
                n_src = ghn_sb
            else:
                n_src = gh_n
            for X in halves:
                nc.vector.tensor_mul(rn[:, X, :], r[:, X, :], n_src[:, X, :])
            for X in halves:
                nc.vector.tensor_add(n_pre[:, X, :], rn[:, X, :], gi_n[:, X, :])
            for X in halves:
                nc.scalar.activation(n[:, X, :], n_pre[:, X, :], AF.Tanh, scale=SC)
            for X in halves:
                nc.vector.tensor_sub(d[:, X, :], h_f[:, X, :], n[:, X, :])
            # z tail, full-width: add -> sigmoid -> mul -> adds
            nc.vector.tensor_add(z_pre[:], gi_z, gh_z[:])
            nc.scalar.activation(z[:], z_pre[:], AF.Sigmoid, scale=SC)
            nc.vector.tensor_mul(zd[:], z[:], d[:])
            # h (bf16) in halves: the next step's matmuls consume k-chunks
            # 0-3 first, so they start as soon as the first half lands
            for X in halves:
                nc.vector.tensor_add(nh_b[:, X, :], n[:, X, :], zd[:, X, :])
            nc.vector.tensor_add(nh_f[:], n[:], zd[:])
            h_b, h_f = nh_b, nh_f

        # ---- encoder ----
        from collections import deque

        # chunk-0 gi pieces drain inside step 0 (mid_fill), after the r/n
        # gh matmuls: those only need h0 + the sync ring's first W_hh
        # slices, so the PE ramps ~4us earlier than if pieces ran first
        piece_q = deque()
        gi_t = gipool.tile([128, 3, MG, GCH * TC], BF16, tag="gi")
        piece_q.extend(gi_pieces(gi_t, wih_e, xenc[:, :, 0 : GCH * TC], gib_e))
        def drain_pieces(k=2):
            for _ in range(k):
                if piece_q:
                    piece_q.popleft()()

        dec_chunks = []
        for s in range(C):
            g = s // GCH
            if s % GCH == 0 and g + 1 < C // GCH:
                gi_next = gipool.tile([128, 3, MG, GCH * TC], BF16, tag="gi")
                piece_q.extend(gi_pieces(
                    gi_next, wih_e,
                    xenc[:, :, (g + 1) * GCH * TC : (g + 2) * GCH * TC],
                    gib_e,
                ))
            # decoder chunks 0-3 fill the encoder steps' idle (the ~3us h
            # chain has no scores work to hide under here); bf16 gi tiles
            # keep 5 chunks alive in SBUF
            if 0 < s <= 6 and s != 5:
                dc = {1: 0, 2: 1, 3: 2, 4: 3, 6: 4}[s]
                gi_d = gipool.tile([128, 3, MG, GCH * TC], BF16, tag="gi")
                piece_q.extend(gi_pieces(
                    gi_d, wih_d,
                    xdec[:, :, dc * GCH * TC : (dc + 1) * GCH * TC], gib_d))
                dec_chunks.append(gi_d)
            gru_step(whh_e, gi_t, s % GCH, ghn_e,
                     mid_fill=(lambda: drain_pieces(6)) if s == 0 else None)
            drain_pieces(4)
            if s == 0:
                # dec weights load during the encoder (DMA is idle here)
                wih_d = wipool.tile([128, KE, 3 * H], BF16, tag="wih")
                nc.scalar.dma_start(wih_d[:], d_wih_d)
                whh_d = whpool.tile([128, KH, 3 * H], F8E3, tag="whh")
                nc.sync.dma_start(whh_d[:], d_whh_d)
            if s % GCH == GCH - 1 and g + 1 < C // GCH:
                gi_t = gi_next

        # output rows are stored step-major (row = s*TC + t, contiguous
        # per block); the host reorders rows to token-major afterwards
        def scores_block(sb, c, slab, par):
            """Scores for step block sb (4 steps) x vocab chunk c."""
            ncols = min(VCH, V - c * VCH)
            ps = ps_sc.tile([128, VCH], F32, tag="ps_sc")
            for k in range(KH):
                nc.tensor.matmul(
                    ps[:, :ncols],
                    hstT[:, k, 4 * sb : 4 * sb + 4, :],
                    slab[:, k, :ncols],
                    start=(k == 0),
                    stop=False if has_outb else (k == KH - 1),
                )
            if has_outb:
                nc.tensor.matmul(
                    ps[:, :ncols], ones_row[:],
                    outb_sb[:, c * VCH : c * VCH + ncols],
                    start=False, stop=True,
                )
            st = stpool.tile([128, VCH], BF16, tag="st")
            nc.scalar.copy(st[:, :ncols], ps[:, :ncols])
            # stores on the ACT HWDGE ring; slab loads stay on SP's
            nc.scalar.dma_start(
                d_scores[128 * sb : 128 * (sb + 1), c * VCH : c * VCH + ncols],
                st[:, :ncols],
            )

        # ---- decoder (scores for the first vocab chunks fill step tails) ----
        N_INTER = 8
        inter_slabs = []
        for c in range(N_INTER):
            slab = spool.tile([128, KH, VCH], BF16, tag="slab")
            nc.sync.dma_start(slab[:], d_ow[c])
            inter_slabs.append(slab)

        pending = deque()
        gi_t = dec_chunks[0]
        gi_ready = deque(dec_chunks[1:])
        for s in range(F):
            g = s // GCH
            if s % GCH == 0 and g + 5 < F // GCH:
                gi_next = gipool.tile([128, 3, MG, GCH * TC], BF16, tag="gi")
                piece_q.extend(gi_pieces(
                    gi_next, wih_d,
                    xdec[:, :, (g + 5) * GCH * TC : (g + 6) * GCH * TC],
                    gib_d,
                ))
                gi_ready.append(gi_next)
            gru_step(whh_d, gi_t, s % GCH, ghn_d, hst_write_step=s)
            drain_pieces()
            if s % GCH == GCH - 1 and g + 1 < F // GCH:
                gi_t = gi_ready.popleft()
            # scores blocks from ALREADY-COMPLETE step blocks fill the step
            # tails; 1.5/step keeps the PE work-bound past the ~3us h chain
            for _ in range(2):
                if pending:
                    sb, c = pending.popleft()
                    scores_block(sb, c, inter_slabs[c], sb + c)
            if s % 4 == 3:
                pending.extend((s // 4, c) for c in range(N_INTER))
        for sb, c in pending:
            scores_block(sb, c, inter_slabs[c], sb + c)

        # ---- remaining vocab projection ----
        for c in range(N_INTER, NV):
            slab = spool.tile([128, KH, VCH], BF16, tag="slab")
            nc.sync.dma_start(slab[:], d_ow[c])
            for sb in range(F // 4):
                scores_block(sb, c, slab, sb + c)

    nc.compile()
    return nc


def _prep_inputs(token_ctx, char_emb_w, enc_W_ih, enc_W_hh, enc_b_ih, enc_b_hh,
                 dec_W_ih, dec_W_hh, dec_b_ih, dec_b_hh, out_W, out_b,
                 in_sent_token_chars, out_chars):
    """Host-side sharding/layout prep. Returns (in_maps, flags, fixup_info)."""
    tcarr = np.asarray(in_sent_token_chars)[0].reshape(T, C, 3)
    chars = tcarr[:, :, 2]
    xt = tcarr[:, :, 1]
    token_ctx = np.asarray(token_ctx)[0]          # [S, H]
    char_emb_w = np.asarray(char_emb_w)           # [V, E]
    out_chars = np.asarray(out_chars)[0]          # [1 + T*F]

    h0 = token_ctx[xt].mean(axis=1).astype(np.float32)      # [T, H]
    x_enc = char_emb_w[chars]                                # [T, C, E]
    gold = out_chars[1 : 1 + T * F].reshape(T, F)
    c0 = out_chars[0]
    c_in = np.concatenate(
        [np.full((T, 1), c0, dtype=gold.dtype), gold[:, :-1]], axis=1
    )                                                        # [T, F]
    x_dec = char_emb_w[c_in]                                 # [T, F, E]

    # shared (replicated) weight layouts; W_hh is fp8 e3m4 scaled x64 (the
    # gate activations undo it), so W_ih/biases are pre-scaled to match
    whhTe = _to_lhsT_layout(np.asarray(enc_W_hh) * WSCALE).astype(npe3m4)
    whhTd = _to_lhsT_layout(np.asarray(dec_W_hh) * WSCALE).astype(npe3m4)
    wihTe = _to_lhsT_layout(np.asarray(enc_W_ih) * WSCALE).astype(npbf16)
    wihTd = _to_lhsT_layout(np.asarray(dec_W_ih) * WSCALE).astype(npbf16)
    owpad = np.zeros((VPAD, H), np.float32)
    owpad[:V] = np.asarray(out_W)
    owT = np.ascontiguousarray(
        owpad.reshape(NV, VCH, KH, 128).transpose(0, 3, 2, 1)
    ).astype(npbf16)                                          # [NV,128,KH,VCH]

    def gate_bias(b_ih, b_hh):
        # gib/ghn live in the x WSCALE PSUM domain; gibrz (r/z fold path) is
        # applied AFTER the activation's scale=1/WSCALE, so it stays unscaled
        b_ih = np.asarray(b_ih); b_hh = np.asarray(b_hh)
        grz = b_ih[: 2 * H] + b_hh[: 2 * H]
        gib = np.concatenate([grz, b_ih[2 * H :]]) * WSCALE
        ghn = b_hh[2 * H :] * WSCALE
        gib_l = np.ascontiguousarray(gib.reshape(24, 128).T).astype(np.float32)
        ghn_l = np.ascontiguousarray(ghn.reshape(MG, 128).T).astype(np.float32)
        grz_l = np.ascontiguousarray(grz.reshape(2 * MG, 128).T).astype(np.float32)
        return gib_l, ghn_l, grz_l, bool(np.any(gib)), bool(np.any(ghn))

    gibE, ghnE, grzE, has_gib_e, has_ghn_e = gate_bias(enc_b_ih, enc_b_hh)
    gibD, ghnD, grzD, has_gib_d, has_ghn_d = gate_bias(dec_b_ih, dec_b_hh)
    out_b = np.asarray(out_b)
    has_outb = bool(np.any(out_b))
    outb_pad = np.zeros((1, VPAD), npbf16)
    outb_pad[0, :V] = out_b.astype(npbf16)

    flags = (has_gib_e, has_ghn_e, has_gib_d, has_ghn_d, has_outb)

    in_maps = []
    for ci in range(NCORES):
        sl = slice(ci * TC, (ci + 1) * TC)
        h0T = np.ascontiguousarray(
            h0[sl].T.reshape(KH, 128, TC).transpose(1, 0, 2)
        )
        # enc ts = c*TC + t (step-major)
        xe = x_enc[sl].transpose(1, 0, 2).reshape(C * TC, E).astype(np.float32)
        xencT = _cols_layout(xe).astype(npbf16)
        # dec ts = s*TC + t (step-major)
        xd = x_dec[sl].transpose(1, 0, 2).reshape(TS, E).astype(np.float32)
        xdecT = _cols_layout(xd).astype(npbf16)
        m = {
            "h0T": h0T, "xencT": xencT, "xdecT": xdecT,
            "whhTe": whhTe, "whhTd": whhTd, "wihTe": wihTe, "wihTd": wihTd,
            "owT": owT,
        }
        if has_gib_e: m["gibE"] = gibE; m["gibrzE"] = grzE
        if has_gib_d: m["gibD"] = gibD; m["gibrzD"] = grzD
        if has_ghn_e: m["ghnE"] = ghnE
        if has_ghn_d: m["ghnD"] = ghnD
        if has_outb: m["outb"] = outb_pad
        in_maps.append(m)

    return in_maps, flags, (gold, c0)


def _eos_fixup(scores, gold, c0):
    """Apply the reference's EOS freeze/pad semantics on the host.
    scores: [T, F, V] (modified in place)."""
    if c0 != EOS and not np.any(gold == EOS):
        return scores
    done0 = c0 == EOS
    for t in range(T):
        hits = np.nonzero(gold[t] == EOS)[0]
        if done0:
            first_done = 0
        elif len(hits):
            first_done = int(hits[0]) + 1
        else:
            continue
        if first_done == 0:
            scores[t, :, :] = 0.0
        elif first_done < F:
            scores[t, first_done:, :] = scores[t, first_done - 1, :]
    return scores


def kernel(**inputs) -> np.ndarray:
    assert int(inputs["max_tokens"]) == T
    assert int(inputs["max_form_len"]) == F
    assert int(inputs["use_teacher_forcing"]) == 1

    in_maps, flags, (gold, c0) = _prep_inputs(
        inputs["token_ctx"], inputs["char_emb_w"],
        inputs["enc_W_ih"], inputs["enc_W_hh"], inputs["enc_b_ih"], inputs["enc_b_hh"],
        inputs["dec_W_ih"], inputs["dec_W_hh"], inputs["dec_b_ih"], inputs["dec_b_hh"],
        inputs["out_W"], inputs["out_b"],
        inputs["in_sent_token_chars"], inputs["out_chars"],
    )

    if flags not in _CACHE:
        _CACHE[flags] = _build_program(flags)
    nc = _CACHE[flags]

    trace = bool(_RUN_OPTS.get("trace"))
    res = run_bass_kernel_spmd(
        nc, in_maps, core_ids=list(range(NCORES)), trace=trace,
        **_RUN_OPTS.get("kwargs", {}),
    )
    _RUN_OPTS["last_result"] = res

    # device rows are step-major per core; reorder to token-major
    slabs = [
        res.results[ci]["scores"].astype(np.float32).reshape(F, TC, V).transpose(1, 0, 2)
        for ci in range(NCORES)
    ]
    out = np.concatenate(slabs, axis=0)  # [T, F, V]
    out = _eos_fixup(out, gold, c0)
    return np.ascontiguousarray(out.reshape(1, T * F, V))


# knobs used by test.py (harness just calls kernel())
_RUN_OPTS = {"trace": False, "kwargs": {}}

